# revision 8
# baseline (speedup 1.0000x reference)
"""Causal self-attention (B=4, T=2048, C=1024, NH=16) on 8 TRN2 NeuronCores.

Sharding: tensor-parallel over heads — 2 heads per core. Each core computes
its slice of qkv (transposed layout), full causal attention for its heads,
and a partial output projection; the host sums the 8 partials and adds b_proj.

Matmuls run in float32r (fp32 bits, reduced-precision PE mode, ~1.6e-4 rel
err) which streams at bf16 speed for free dims >= 256.

Layout notes:
 - qkv is computed transposed (qT/kT/vT: [dim, tok]) so scores can be formed
   as scoresT[k, q] = kT.T @ qT with d on partitions; softmax runs over the
   partition (k) axis using exp on ACT, a ones-column in the p@v matmul for
   the denominator, and a K=1 broadcast matmul for the reciprocal.
 - x is transposed on the host once (layout prep during sharding) so qkv
   needs no on-device transposes; v is re-transposed to natural layout on PE.
"""

import sys

import numpy as np

try:
    import concourse.bass as bass
except ImportError:  # grading container may not have it on sys.path
    sys.path.insert(0, "/opt/trn_rl_repo")
    import concourse.bass as bass

from contextlib import ExitStack

import concourse.mybir as mybir
import concourse.tile as tile
from concourse.bass_utils import run_bass_kernel_spmd


B, T, C, NH, HD = 4, 2048, 1024, 16, 64
N_CORES = 8
HPC = NH // N_CORES  # heads per core = 2
DPC = HPC * HD  # dims per core = 128
BT = B * T  # 8192
QCH = 512  # q-chunk (moving free dim)
KCH = 128  # k-chunk (contraction tile)
TCH = 512  # token chunk for qkv
F32 = mybir.dt.float32
F32R = mybir.dt.float32r
AF = mybir.ActivationFunctionType


def _r(ap):
    return ap.bitcast(F32R)


def _split_multi_waits(nc):
    """Walrus in this container accepts only ONE sync wait per instruction.
    Hoist extra waits onto same-engine NoOps inserted just before."""
    n = 0
    for f in nc.m.functions:
        for b in f.blocks:
            insts = b.instructions
            if not any(
                i.sync_info is not None
                and i.sync_info.on_wait
                and len(i.sync_info.on_wait) > 1
                for i in insts
            ):
                continue
            new = []
            for ins in insts:
                si = ins.sync_info
                if si is not None and si.on_wait and len(si.on_wait) > 1:
                    waits = list(si.on_wait)
                    for w in waits[:-1]:
                        nop = mybir.InstNoOp(
                            name=f"{ins.name}-ws{n}", ins=[], outs=[]
                        )
                        nop.engine = ins.engine
                        nop.bass_nofuse = True
                        nop.sync_info = mybir.SyncInfo(on_wait=[w], on_update=[])
                        if ins.debug is not None:
                            nop.debug = ins.debug
                        new.append(nop)
                        n += 1
                    ins.sync_info = mybir.SyncInfo(
                        on_wait=[waits[-1]], on_update=list(si.on_update or [])
                    )
                new.append(ins)
            b.instructions = new
    return n


def build_kernel():
    nc = bass.Bass("TRN2", target_bir_lowering=False, debug=False, num_devices=N_CORES)
    xT_d = nc.dram_tensor("xT", [C, BT], F32R, kind="ExternalInput")
    wc_d = nc.dram_tensor("wc", [C, 3 * DPC], F32R, kind="ExternalInput")
    bc_d = nc.dram_tensor("bc", [3, DPC, 1], F32, kind="ExternalInput")
    wp_d = nc.dram_tensor("wp", [DPC, C], F32R, kind="ExternalInput")
    out_d = nc.dram_tensor("out", [BT, C], F32, kind="ExternalOutput")

    with tile.TileContext(nc) as tc, ExitStack() as ctx:
        consts = ctx.enter_context(tc.tile_pool(name="consts", bufs=1))
        xpool = ctx.enter_context(tc.tile_pool(name="x", bufs=16))
        qkvp = ctx.enter_context(tc.tile_pool(name="qkv", bufs=2))
        vexp = ctx.enter_context(tc.tile_pool(name="vext", bufs=2))
        ytp = ctx.enter_context(tc.tile_pool(name="yt", bufs=2))
        expp = ctx.enter_context(tc.tile_pool(name="expt", bufs=4))
        smallp = ctx.enter_context(tc.tile_pool(name="small", bufs=4))
        outp = ctx.enter_context(tc.tile_pool(name="outt", bufs=4))
        ps_mm = ctx.enter_context(tc.tile_pool(name="ps_mm", bufs=4, space="PSUM"))
        ps_acc = ctx.enter_context(tc.tile_pool(name="ps_acc", bufs=2, space="PSUM"))

        # [128, 64] tile holding I64 in partitions 0-63 and again in 64-127,
        # so each head's vT slice has an identity at its own base partition.
        ident = consts.tile([128, 64], F32)
        nc.gpsimd.memset(ident, 0.0)
        for half in range(2):
            nc.gpsimd.affine_select(
                out=ident[64 * half : 64 * half + 64, :],
                in_=ident[64 * half : 64 * half + 64, :],
                compare_op=mybir.AluOpType.not_equal,
                fill=1.0,
                base=0,
                pattern=[[-1, 64]],
                channel_multiplier=1,
            )
        ones_row = consts.tile([1, 64], F32R)
        nc.vector.memset(ones_row.bitcast(F32), 1.0)

        # weights: wc [1024, 384] -> [128, 8, 384] (kc chunks on free dim)
        w_sb = consts.tile([128, 8, 3 * DPC], F32R)
        nc.sync.dma_start(
            out=w_sb, in_=wc_d.ap().rearrange("(kc p) c -> p kc c", p=128)
        )
        wp_sb = consts.tile([128, C], F32R)
        nc.sync.dma_start(out=wp_sb, in_=wp_d.ap())
        bc_sb = consts.tile([128, 3], F32)
        nc.sync.dma_start(out=bc_sb, in_=bc_d.ap().rearrange("g p one -> p (g one)"))

        NKC = C // 128  # 8 contraction chunks for qkv
        NTC = T // TCH  # 4 token chunks per batch
        NQC = T // QCH  # 4 q-chunks per batch (per head)
        NVC = T // 128  # 16 v chunks per batch

        for b in range(B):
            t0 = b * T
            # ---- qkv (transposed): qT/kT/vT [128, T] for this batch ----
            qT = qkvp.tile([128, T], F32R, tag="qT")
            kT = qkvp.tile([128, T], F32R, tag="kT")
            vT = qkvp.tile([128, T], F32, tag="vT")
            dest = [qT, kT, vT]
            for tcb in range(NTC):
                xts = []
                for kc in range(NKC):
                    xt = xpool.tile([128, TCH], F32R, tag="xt")
                    nc.sync.dma_start(
                        out=xt,
                        in_=xT_d.ap()[
                            kc * 128 : (kc + 1) * 128,
                            t0 + tcb * TCH : t0 + (tcb + 1) * TCH,
                        ],
                    )
                    xts.append(xt)
                for g in range(3):
                    ps = ps_mm.tile([128, TCH], F32, tag="mm")
                    for kc in range(NKC):
                        nc.tensor.matmul(
                            ps,
                            w_sb[:, kc, g * 128 : (g + 1) * 128],
                            xts[kc],
                            start=(kc == 0),
                            stop=(kc == NKC - 1),
                        )
                    # psum -> sbuf with bias add (b_attn slice, per-partition)
                    nc.scalar.activation(
                        dest[g][:, tcb * TCH : (tcb + 1) * TCH],
                        ps,
                        AF.Identity,
                        bias=bc_sb[:, g : g + 1],
                    )

            # ---- v back to natural layout, with ones column: [128, 65] ----
            vex = vexp.tile([128, HPC, NVC, 65], F32R, tag="vex")
            nc.vector.memset(vex[:, :, :, 64:65].bitcast(F32), 1.0)
            for h in range(HPC):
                for j in range(NVC):
                    pt = ps_mm.tile([128, 64], F32, tag="mm")
                    nc.tensor.transpose(
                        pt,
                        vT[64 * h : 64 * h + 64, j * 128 : (j + 1) * 128],
                        ident[64 * h : 64 * h + 64, :],
                    )
                    nc.vector.tensor_copy(vex[:, h, j, 0:64], pt)

            # ---- causal attention, transposed-scores flash style ----
            yT = ytp.tile([128, T], F32R, tag="yT")
            for h in range(HPC):
                qTh = qT[64 * h : 64 * h + 64, :]
                kTh = kT[64 * h : 64 * h + 64, :]
                for qc in range(NQC):
                    q0 = qc * QCH
                    nk = (q0 + QCH) // KCH
                    yt_ps = ps_acc.tile([65, QCH], F32, tag="yt")
                    for j in range(nk):
                        k0 = j * KCH
                        qlo = max(0, k0 - q0)
                        sc = ps_mm.tile([128, QCH], F32, tag="mm")
                        nc.tensor.matmul(
                            sc[:, qlo:QCH],
                            kTh[:, k0 : k0 + KCH],
                            qTh[:, q0 + qlo : q0 + QCH],
                            start=True,
                            stop=True,
                        )
                        ex = expp.tile([128, QCH], F32R, tag="ex")
                        nc.scalar.activation(
                            ex[:, qlo:QCH], sc[:, qlo:QCH], AF.Exp, scale=0.125
                        )
                        if qlo > 0 or k0 == q0:
                            # diagonal 128-wide block: zero where k > q
                            nc.gpsimd.affine_select(
                                out=ex[:, qlo : qlo + 128],
                                in_=ex[:, qlo : qlo + 128],
                                compare_op=mybir.AluOpType.is_ge,
                                fill=0.0,
                                base=0,
                                pattern=[[1, 128]],
                                channel_multiplier=-1,
                            )
                        nc.tensor.matmul(
                            yt_ps[:, qlo:QCH],
                            vex[:, h, j, :],
                            ex[:, qlo:QCH],
                            start=(j == 0),
                            stop=(j == nk - 1),
                        )
                    # normalize: recip of ones-row sums, broadcast via K=1 matmul
                    recip = smallp.tile([1, QCH], F32R, tag="recip")
                    with nc.allow_low_precision(reason="f32r recip for bcast matmul"):
                        nc.vector.reciprocal(recip, yt_ps[64:65, :])
                    bc_ps = ps_mm.tile([64, QCH], F32, tag="mm")
                    nc.tensor.matmul(
                        bc_ps, ones_row, recip, start=True, stop=True
                    )
                    bc_sb2 = smallp.tile([64, QCH], F32, tag="bcast")
                    nc.scalar.copy(bc_sb2, bc_ps)
                    nc.vector.tensor_mul(
                        yT[64 * h : 64 * h + 64, q0 : q0 + QCH],
                        yt_ps[0:64, :],
                        bc_sb2,
                    )

            # ---- output projection (partial over this core's 128 dims) ----
            for tcb in range(T // 128):
                for g in range(2):
                    ps = ps_mm.tile([128, 512], F32, tag="mm")
                    nc.tensor.matmul(
                        ps,
                        yT[:, tcb * 128 : (tcb + 1) * 128],
                        wp_sb[:, g * 512 : (g + 1) * 512],
                        start=True,
                        stop=True,
                    )
                    ot = outp.tile([128, 512], F32, tag="ot")
                    if g == 0:
                        nc.vector.tensor_copy(ot, ps)
                    else:
                        nc.scalar.copy(ot, ps)
                    nc.sync.dma_start(
                        out=out_d.ap()[
                            t0 + tcb * 128 : t0 + (tcb + 1) * 128,
                            g * 512 : (g + 1) * 512,
                        ],
                        in_=ot,
                    )

    _split_multi_waits(nc)
    return nc


_NC_CACHE = None


def _get_nc():
    global _NC_CACHE
    if _NC_CACHE is None:
        _NC_CACHE = build_kernel()
    return _NC_CACHE


def kernel_with_results(x, W_attn, b_attn, W_proj, b_proj, trace=False):
    x = np.asarray(x, dtype=np.float32)
    W_attn = np.asarray(W_attn, dtype=np.float32)
    b_attn = np.asarray(b_attn, dtype=np.float32)
    W_proj = np.asarray(W_proj, dtype=np.float32)
    b_proj = np.asarray(b_proj, dtype=np.float32)

    xT = np.ascontiguousarray(x.reshape(BT, C).T)  # [C, BT]
    in_maps = []
    for c in range(N_CORES):
        lo = c * DPC
        wc = np.ascontiguousarray(
            np.concatenate(
                [
                    W_attn[:, lo : lo + DPC],
                    W_attn[:, C + lo : C + lo + DPC],
                    W_attn[:, 2 * C + lo : 2 * C + lo + DPC],
                ],
                axis=1,
            )
        )
        bc = np.ascontiguousarray(
            np.stack(
                [
                    b_attn[lo : lo + DPC],
                    b_attn[C + lo : C + lo + DPC],
                    b_attn[2 * C + lo : 2 * C + lo + DPC],
                ]
            ).reshape(3, DPC, 1)
        )
        wp = np.ascontiguousarray(W_proj[lo : lo + DPC, :])
        in_maps.append({"xT": xT, "wc": wc, "bc": bc, "wp": wp})

    nc = _get_nc()
    res = run_bass_kernel_spmd(
        nc, in_maps, core_ids=list(range(N_CORES)), trace=trace
    )
    acc = np.zeros((BT, C), dtype=np.float64)
    for c in range(N_CORES):
        acc += res.results[c]["out"].astype(np.float64)
    out = (acc + b_proj.astype(np.float64)).astype(np.float32)
    return out.reshape(B, T, C), res


def kernel(x, W_attn, b_attn, W_proj, b_proj):
    out, _ = kernel_with_results(x, W_attn, b_attn, W_proj, b_proj)
    return out


# revision 12
# speedup vs baseline: 1.0803x; 1.0803x over previous
"""Causal self-attention (B=4, T=2048, C=1024, NH=16) on 8 TRN2 NeuronCores.

Sharding: tensor-parallel over heads — 2 heads per core. Each core computes
its slice of qkv (transposed layout), full causal attention for its heads,
and a partial output projection; the host sums the 8 partials and adds b_proj.

Matmuls run in float32r (fp32 bits, reduced-precision PE mode, ~1.6e-4 rel
err) which streams at bf16 speed for free dims >= 256.

Layout notes:
 - qkv is computed transposed (qT/kT/vT: [dim, tok]) so scores can be formed
   as scoresT[k, q] = kT.T @ qT with d on partitions; softmax runs over the
   partition (k) axis using exp on ACT, a ones-column in the p@v matmul for
   the denominator, and a K=1 broadcast matmul for the reciprocal.
 - x is transposed on the host once (layout prep during sharding) so qkv
   needs no on-device transposes; v is re-transposed to natural layout on PE.
"""

import sys

import numpy as np

try:
    import concourse.bass as bass
except ImportError:  # grading container may not have it on sys.path
    sys.path.insert(0, "/opt/trn_rl_repo")
    import concourse.bass as bass

from contextlib import ExitStack

import concourse.mybir as mybir
import concourse.tile as tile
from concourse.bass_utils import run_bass_kernel_spmd


B, T, C, NH, HD = 4, 2048, 1024, 16, 64
N_CORES = 8
HPC = NH // N_CORES  # heads per core = 2
DPC = HPC * HD  # dims per core = 128
BT = B * T  # 8192
QCH = 512  # q-chunk (moving free dim)
KCH = 128  # k-chunk (contraction tile)
TCH = 512  # token chunk for qkv
F32 = mybir.dt.float32
F32R = mybir.dt.float32r
AF = mybir.ActivationFunctionType


def _r(ap):
    return ap.bitcast(F32R)


def _act_reciprocal(nc, out, in_):
    """Reciprocal on the scalar engine (~430ns for [1,512] vs ~3.3us for
    nc.vector.reciprocal's Newton chain). bass blocks AF.Reciprocal behind an
    accuracy warning; the spline is good to ~1e-5 rel which is far below this
    kernel's f32r noise floor, so emit the instruction directly."""
    eng = nc.scalar
    ins = [
        eng.lower_ap(in_),
        mybir.ImmediateValue(dtype=mybir.dt.float32, value=0.0),
        mybir.ImmediateValue(dtype=mybir.dt.float32, value=1.0),
        mybir.ImmediateValue(dtype=mybir.dt.float32, value=0.0),
    ]
    return eng.add_instruction(
        mybir.InstActivation(
            name=nc.get_next_instruction_name(),
            func=AF.Reciprocal,
            ins=ins,
            outs=[eng.lower_ap(out)],
        )
    )


def _split_multi_waits(nc):
    """Walrus in this container accepts only ONE sync wait per instruction.
    Hoist extra waits onto same-engine NoOps inserted just before."""
    n = 0
    for f in nc.m.functions:
        for b in f.blocks:
            insts = b.instructions
            if not any(
                i.sync_info is not None
                and i.sync_info.on_wait
                and len(i.sync_info.on_wait) > 1
                for i in insts
            ):
                continue
            new = []
            for ins in insts:
                si = ins.sync_info
                if si is not None and si.on_wait and len(si.on_wait) > 1:
                    waits = list(si.on_wait)
                    for w in waits[:-1]:
                        nop = mybir.InstNoOp(
                            name=f"{ins.name}-ws{n}", ins=[], outs=[]
                        )
                        nop.engine = ins.engine
                        nop.bass_nofuse = True
                        nop.sync_info = mybir.SyncInfo(on_wait=[w], on_update=[])
                        if ins.debug is not None:
                            nop.debug = ins.debug
                        new.append(nop)
                        n += 1
                    ins.sync_info = mybir.SyncInfo(
                        on_wait=[waits[-1]], on_update=list(si.on_update or [])
                    )
                new.append(ins)
            b.instructions = new
    return n


def build_kernel():
    nc = bass.Bass("TRN2", target_bir_lowering=False, debug=False, num_devices=N_CORES)
    xT_d = nc.dram_tensor("xT", [C, BT], F32R, kind="ExternalInput")
    wc_d = nc.dram_tensor("wc", [C, 3 * DPC], F32R, kind="ExternalInput")
    bc_d = nc.dram_tensor("bc", [3, DPC, 1], F32, kind="ExternalInput")
    wp_d = nc.dram_tensor("wp", [DPC, C], F32R, kind="ExternalInput")
    out_d = nc.dram_tensor("out", [BT, C], F32, kind="ExternalOutput")

    with tile.TileContext(nc) as tc, ExitStack() as ctx:
        consts = ctx.enter_context(tc.tile_pool(name="consts", bufs=1))
        xpool = ctx.enter_context(tc.tile_pool(name="x", bufs=16))
        qkvp = ctx.enter_context(tc.tile_pool(name="qkv", bufs=2))
        vexp = ctx.enter_context(tc.tile_pool(name="vext", bufs=2))
        ytp = ctx.enter_context(tc.tile_pool(name="yt", bufs=2))
        expp = ctx.enter_context(tc.tile_pool(name="expt", bufs=6))
        smallp = ctx.enter_context(tc.tile_pool(name="small", bufs=4))
        outp = ctx.enter_context(tc.tile_pool(name="outt", bufs=4))
        ps_acc = ctx.enter_context(tc.tile_pool(name="ps_acc", bufs=6, space="PSUM"))
        ps_sc = ctx.enter_context(tc.tile_pool(name="ps_sc", bufs=2, space="PSUM"))

        # [128, 64] tile holding I64 in partitions 0-63 and again in 64-127,
        # so each head's vT slice has an identity at its own base partition.
        ident = consts.tile([128, 64], F32)
        nc.gpsimd.memset(ident, 0.0)
        for half in range(2):
            nc.gpsimd.affine_select(
                out=ident[64 * half : 64 * half + 64, :],
                in_=ident[64 * half : 64 * half + 64, :],
                compare_op=mybir.AluOpType.not_equal,
                fill=1.0,
                base=0,
                pattern=[[-1, 64]],
                channel_multiplier=1,
            )
        ones_row = consts.tile([1, 64], F32R)
        nc.vector.memset(ones_row.bitcast(F32), 1.0)

        # weights: wc [1024, 384] -> [128, 8, 384] (kc chunks on free dim)
        w_sb = consts.tile([128, 8, 3 * DPC], F32R)
        nc.sync.dma_start(
            out=w_sb, in_=wc_d.ap().rearrange("(kc p) c -> p kc c", p=128)
        )
        wp_sb = consts.tile([128, C], F32R)
        nc.sync.dma_start(out=wp_sb, in_=wp_d.ap())
        bc_sb = consts.tile([128, 3], F32)
        nc.sync.dma_start(out=bc_sb, in_=bc_d.ap().rearrange("g p one -> p (g one)"))

        NKC = C // 128  # 8 contraction chunks for qkv
        NTC = T // TCH  # 4 token chunks per batch
        NQC = T // QCH  # 4 q-chunks per batch (per head)
        NVC = T // 128  # 16 v chunks per batch

        for b in range(B):
            t0 = b * T
            # ---- qkv (transposed): qT/kT/vT [128, T] for this batch ----
            qT = qkvp.tile([128, T], F32R, tag="qT")
            kT = qkvp.tile([128, T], F32R, tag="kT")
            vT = qkvp.tile([128, T], F32, tag="vT")
            dest = [qT, kT, vT]
            for tcb in range(NTC):
                xts = []
                for kc in range(NKC):
                    xt = xpool.tile([128, TCH], F32R, tag="xt")
                    nc.sync.dma_start(
                        out=xt,
                        in_=xT_d.ap()[
                            kc * 128 : (kc + 1) * 128,
                            t0 + tcb * TCH : t0 + (tcb + 1) * TCH,
                        ],
                    )
                    xts.append(xt)
                for g in range(3):
                    ps = ps_acc.tile([128, TCH], F32, tag="acc")
                    for kc in range(NKC):
                        nc.tensor.matmul(
                            ps,
                            w_sb[:, kc, g * 128 : (g + 1) * 128],
                            xts[kc],
                            start=(kc == 0),
                            stop=(kc == NKC - 1),
                        )
                    # psum -> sbuf with bias add (b_attn slice, per-partition)
                    nc.scalar.activation(
                        dest[g][:, tcb * TCH : (tcb + 1) * TCH],
                        ps,
                        AF.Identity,
                        bias=bc_sb[:, g : g + 1],
                    )

            # ---- v back to natural layout, with ones column: [128, 65] ----
            vex = vexp.tile([128, HPC, NVC, 65], F32R, tag="vex")
            nc.vector.memset(vex[:, :, :, 64:65].bitcast(F32), 1.0)
            for h in range(HPC):
                for j in range(NVC):
                    pt = ps_sc.tile([128, 64], F32, tag="sc")
                    nc.tensor.transpose(
                        pt,
                        vT[64 * h : 64 * h + 64, j * 128 : (j + 1) * 128],
                        ident[64 * h : 64 * h + 64, :],
                    )
                    nc.vector.tensor_copy(vex[:, h, j, 0:64], pt)

            # ---- causal attention, transposed-scores flash style ----
            # qc-pair blocking with both heads interleaved per k-chunk:
            # keeps up to 4 independent (h, qc) chains in flight so PE
            # streams matmuls back-to-back while ACT runs the exps, and the
            # two heads' K=64 score matmuls land in separate PE row groups
            # (base partitions 0/64) for array-level concurrency.
            yT = ytp.tile([128, T], F32R, tag="yT")
            for qcs in ((0, 1), (2, 3)):
                yts = {}
                for h in range(HPC):
                    for qc in qcs:
                        yts[(h, qc)] = ps_acc.tile(
                            [65, QCH], F32, name=f"yt_h{h}q{qc}", tag="acc"
                        )
                jmax = (qcs[-1] * QCH + QCH) // KCH
                for j in range(jmax):
                    k0 = j * KCH
                    for h in range(HPC):
                        qTh = qT[64 * h : 64 * h + 64, :]
                        kTh = kT[64 * h : 64 * h + 64, :]
                        for qc in qcs:
                            q0 = qc * QCH
                            if k0 >= q0 + QCH:
                                continue
                            nk = (q0 + QCH) // KCH
                            qlo = max(0, k0 - q0)
                            sc = ps_sc.tile([128, QCH], F32, tag="sc")
                            nc.tensor.matmul(
                                sc[:, qlo:QCH],
                                kTh[:, k0 : k0 + KCH],
                                qTh[:, q0 + qlo : q0 + QCH],
                                start=True,
                                stop=True,
                            )
                            ex = expp.tile([128, QCH], F32R, tag="ex")
                            nc.scalar.activation(
                                ex[:, qlo:QCH], sc[:, qlo:QCH], AF.Exp, scale=0.125
                            )
                            if k0 >= q0:
                                # diagonal 128-wide block: zero where k > q
                                nc.gpsimd.affine_select(
                                    out=ex[:, qlo : qlo + 128],
                                    in_=ex[:, qlo : qlo + 128],
                                    compare_op=mybir.AluOpType.is_ge,
                                    fill=0.0,
                                    base=0,
                                    pattern=[[1, 128]],
                                    channel_multiplier=-1,
                                )
                            nc.tensor.matmul(
                                yts[(h, qc)][:, qlo:QCH],
                                vex[:, h, j, :],
                                ex[:, qlo:QCH],
                                start=(j == 0),
                                stop=(j == nk - 1),
                            )
                for (h, qc), yt_ps in yts.items():
                    q0 = qc * QCH
                    # normalize: 1/sums on ACT, broadcast via K=1 matmul
                    recip = smallp.tile([1, QCH], F32R, tag="recip")
                    _act_reciprocal(nc, recip, yt_ps[64:65, :])
                    bc_ps = ps_sc.tile([64, QCH], F32, tag="sc")
                    nc.tensor.matmul(bc_ps, ones_row, recip, start=True, stop=True)
                    bc_sb2 = smallp.tile([64, QCH], F32, tag="bcast")
                    nc.vector.tensor_copy(bc_sb2, bc_ps)
                    nc.vector.tensor_mul(
                        yT[64 * h : 64 * h + 64, q0 : q0 + QCH],
                        yt_ps[0:64, :],
                        bc_sb2,
                    )

            # ---- output projection (partial over this core's 128 dims) ----
            for tcb in range(T // 128):
                for g in range(2):
                    ps = ps_acc.tile([128, 512], F32, tag="acc")
                    nc.tensor.matmul(
                        ps,
                        yT[:, tcb * 128 : (tcb + 1) * 128],
                        wp_sb[:, g * 512 : (g + 1) * 512],
                        start=True,
                        stop=True,
                    )
                    ot = outp.tile([128, 512], F32, tag="ot")
                    if g == 0:
                        nc.vector.tensor_copy(ot, ps)
                    else:
                        nc.scalar.copy(ot, ps)
                    nc.sync.dma_start(
                        out=out_d.ap()[
                            t0 + tcb * 128 : t0 + (tcb + 1) * 128,
                            g * 512 : (g + 1) * 512,
                        ],
                        in_=ot,
                    )

    _split_multi_waits(nc)
    return nc


_NC_CACHE = None


def _get_nc():
    global _NC_CACHE
    if _NC_CACHE is None:
        _NC_CACHE = build_kernel()
    return _NC_CACHE


def kernel_with_results(x, W_attn, b_attn, W_proj, b_proj, trace=False):
    x = np.asarray(x, dtype=np.float32)
    W_attn = np.asarray(W_attn, dtype=np.float32)
    b_attn = np.asarray(b_attn, dtype=np.float32)
    W_proj = np.asarray(W_proj, dtype=np.float32)
    b_proj = np.asarray(b_proj, dtype=np.float32)

    xT = np.ascontiguousarray(x.reshape(BT, C).T)  # [C, BT]
    in_maps = []
    for c in range(N_CORES):
        lo = c * DPC
        wc = np.ascontiguousarray(
            np.concatenate(
                [
                    W_attn[:, lo : lo + DPC],
                    W_attn[:, C + lo : C + lo + DPC],
                    W_attn[:, 2 * C + lo : 2 * C + lo + DPC],
                ],
                axis=1,
            )
        )
        bc = np.ascontiguousarray(
            np.stack(
                [
                    b_attn[lo : lo + DPC],
                    b_attn[C + lo : C + lo + DPC],
                    b_attn[2 * C + lo : 2 * C + lo + DPC],
                ]
            ).reshape(3, DPC, 1)
        )
        wp = np.ascontiguousarray(W_proj[lo : lo + DPC, :])
        in_maps.append({"xT": xT, "wc": wc, "bc": bc, "wp": wp})

    nc = _get_nc()
    res = run_bass_kernel_spmd(
        nc, in_maps, core_ids=list(range(N_CORES)), trace=trace
    )
    acc = np.zeros((BT, C), dtype=np.float64)
    for c in range(N_CORES):
        acc += res.results[c]["out"].astype(np.float64)
    out = (acc + b_proj.astype(np.float64)).astype(np.float32)
    return out.reshape(B, T, C), res


def kernel(x, W_attn, b_attn, W_proj, b_proj):
    out, _ = kernel_with_results(x, W_attn, b_attn, W_proj, b_proj)
    return out


# revision 17
# speedup vs baseline: 1.0944x; 1.0131x over previous
"""Causal self-attention (B=4, T=2048, C=1024, NH=16) on 8 TRN2 NeuronCores.

Sharding: tensor-parallel over heads — 2 heads per core. Each core computes
its slice of qkv (transposed layout), full causal attention for its heads,
and a partial output projection; the host sums the 8 partials and adds b_proj.

Matmuls run in float32r (fp32 bits, reduced-precision PE mode, ~1.6e-4 rel
err) which streams at bf16 speed for free dims >= 256.

Layout notes:
 - qkv is computed transposed (qT/kT/vT: [dim, tok]) so scores can be formed
   as scoresT[k, q] = kT.T @ qT with d on partitions; softmax runs over the
   partition (k) axis using exp on ACT, a ones-column in the p@v matmul for
   the denominator, and a K=1 broadcast matmul for the reciprocal.
 - x is transposed on the host once (layout prep during sharding) so qkv
   needs no on-device transposes; v is re-transposed to natural layout on PE.
"""

import sys

import numpy as np

try:
    import concourse.bass as bass
except ImportError:  # grading container may not have it on sys.path
    sys.path.insert(0, "/opt/trn_rl_repo")
    import concourse.bass as bass

from contextlib import ExitStack

import concourse.mybir as mybir
import concourse.tile as tile
from concourse.bass_utils import run_bass_kernel_spmd


B, T, C, NH, HD = 4, 2048, 1024, 16, 64
N_CORES = 8
HPC = NH // N_CORES  # heads per core = 2
DPC = HPC * HD  # dims per core = 128
BT = B * T  # 8192
QCH = 512  # q-chunk (moving free dim)
KCH = 128  # k-chunk (contraction tile)
TCH = 512  # token chunk for qkv
F32 = mybir.dt.float32
F32R = mybir.dt.float32r
AF = mybir.ActivationFunctionType


def _r(ap):
    return ap.bitcast(F32R)


def _act_reciprocal(nc, out, in_):
    """Reciprocal on the scalar engine (~430ns for [1,512] vs ~3.3us for
    nc.vector.reciprocal's Newton chain). bass blocks AF.Reciprocal behind an
    accuracy warning; the spline is good to ~1e-5 rel which is far below this
    kernel's f32r noise floor, so emit the instruction directly."""
    eng = nc.scalar
    ins = [
        eng.lower_ap(in_),
        mybir.ImmediateValue(dtype=mybir.dt.float32, value=0.0),
        mybir.ImmediateValue(dtype=mybir.dt.float32, value=1.0),
        mybir.ImmediateValue(dtype=mybir.dt.float32, value=0.0),
    ]
    return eng.add_instruction(
        mybir.InstActivation(
            name=nc.get_next_instruction_name(),
            func=AF.Reciprocal,
            ins=ins,
            outs=[eng.lower_ap(out)],
        )
    )


def _split_multi_waits(nc):
    """Walrus in this container accepts only ONE sync wait per instruction.
    Hoist extra waits onto same-engine NoOps inserted just before."""
    n = 0
    for f in nc.m.functions:
        for b in f.blocks:
            insts = b.instructions
            if not any(
                i.sync_info is not None
                and i.sync_info.on_wait
                and len(i.sync_info.on_wait) > 1
                for i in insts
            ):
                continue
            new = []
            for ins in insts:
                si = ins.sync_info
                if si is not None and si.on_wait and len(si.on_wait) > 1:
                    waits = list(si.on_wait)
                    for w in waits[:-1]:
                        nop = mybir.InstNoOp(
                            name=f"{ins.name}-ws{n}", ins=[], outs=[]
                        )
                        nop.engine = ins.engine
                        nop.bass_nofuse = True
                        nop.sync_info = mybir.SyncInfo(on_wait=[w], on_update=[])
                        if ins.debug is not None:
                            nop.debug = ins.debug
                        new.append(nop)
                        n += 1
                    ins.sync_info = mybir.SyncInfo(
                        on_wait=[waits[-1]], on_update=list(si.on_update or [])
                    )
                new.append(ins)
            b.instructions = new
    return n


def build_kernel():
    nc = bass.Bass("TRN2", target_bir_lowering=False, debug=False, num_devices=N_CORES)
    xT_d = nc.dram_tensor("xT", [C, BT], F32R, kind="ExternalInput")
    wc_d = nc.dram_tensor("wc", [C, 3 * DPC], F32R, kind="ExternalInput")
    bc_d = nc.dram_tensor("bc", [3, DPC, 1], F32, kind="ExternalInput")
    wp_d = nc.dram_tensor("wp", [DPC, C], F32R, kind="ExternalInput")
    out_d = nc.dram_tensor("out", [BT, C], F32, kind="ExternalOutput")

    with tile.TileContext(nc) as tc, ExitStack() as ctx:
        consts = ctx.enter_context(tc.tile_pool(name="consts", bufs=1))
        xpool = ctx.enter_context(tc.tile_pool(name="x", bufs=16))
        qkvp = ctx.enter_context(tc.tile_pool(name="qkv", bufs=2))
        vexp = ctx.enter_context(tc.tile_pool(name="vext", bufs=2))
        ytp = ctx.enter_context(tc.tile_pool(name="yt", bufs=2))
        expp = ctx.enter_context(tc.tile_pool(name="expt", bufs=6))
        smallp = ctx.enter_context(tc.tile_pool(name="small", bufs=4))
        outp = ctx.enter_context(tc.tile_pool(name="outt", bufs=4))
        ps_acc = ctx.enter_context(tc.tile_pool(name="ps_acc", bufs=6, space="PSUM"))
        ps_sc = ctx.enter_context(tc.tile_pool(name="ps_sc", bufs=2, space="PSUM"))

        # [128, 64] tile holding I64 in partitions 0-63 and again in 64-127,
        # so each head's vT slice has an identity at its own base partition.
        ident = consts.tile([128, 64], F32)
        nc.gpsimd.memset(ident, 0.0)
        for half in range(2):
            nc.gpsimd.affine_select(
                out=ident[64 * half : 64 * half + 64, :],
                in_=ident[64 * half : 64 * half + 64, :],
                compare_op=mybir.AluOpType.not_equal,
                fill=1.0,
                base=0,
                pattern=[[-1, 64]],
                channel_multiplier=1,
            )
        ones_row = consts.tile([1, 64], F32R)
        nc.vector.memset(ones_row.bitcast(F32), 1.0)

        # weights: wc [1024, 384] -> [128, 8, 384] (kc chunks on free dim)
        w_sb = consts.tile([128, 8, 3 * DPC], F32R)
        nc.sync.dma_start(
            out=w_sb, in_=wc_d.ap().rearrange("(kc p) c -> p kc c", p=128)
        )
        wp_sb = consts.tile([128, C], F32R)
        nc.sync.dma_start(out=wp_sb, in_=wp_d.ap())
        bc_sb = consts.tile([128, 3], F32)
        nc.sync.dma_start(out=bc_sb, in_=bc_d.ap().rearrange("g p one -> p (g one)"))

        NKC = C // 128  # 8 contraction chunks for qkv
        NTC = T // TCH  # 4 token chunks per batch
        NQC = T // QCH  # 4 q-chunks per batch (per head)
        NVC = T // 128  # 16 v chunks per batch

        for b in range(B):
            t0 = b * T
            # ---- qkv (transposed): qT/kT/vT [128, T] for this batch ----
            qT = qkvp.tile([128, T], F32R, tag="qT")
            kT = qkvp.tile([128, T], F32R, tag="kT")
            vT = qkvp.tile([128, T], F32, tag="vT")
            dest = [qT, kT, vT]
            for tcb in range(NTC):
                xts = []
                for kc in range(NKC):
                    xt = xpool.tile([128, TCH], F32R, tag="xt")
                    nc.sync.dma_start(
                        out=xt,
                        in_=xT_d.ap()[
                            kc * 128 : (kc + 1) * 128,
                            t0 + tcb * TCH : t0 + (tcb + 1) * TCH,
                        ],
                    )
                    xts.append(xt)
                for g in range(3):
                    ps = ps_acc.tile([128, TCH], F32, tag="acc")
                    for kc in range(NKC):
                        nc.tensor.matmul(
                            ps,
                            w_sb[:, kc, g * 128 : (g + 1) * 128],
                            xts[kc],
                            start=(kc == 0),
                            stop=(kc == NKC - 1),
                        )
                    # psum -> sbuf with bias add (b_attn slice, per-partition);
                    # on DVE to keep ACT free for the attention exps
                    nc.vector.tensor_scalar_add(
                        dest[g][:, tcb * TCH : (tcb + 1) * TCH],
                        ps,
                        bc_sb[:, g : g + 1],
                    )

            # ---- v back to natural layout, with ones column: [128, 65] ----
            vex = vexp.tile([128, HPC, NVC, 65], F32R, tag="vex")
            nc.vector.memset(vex[:, :, :, 64:65].bitcast(F32), 1.0)
            for h in range(HPC):
                for j in range(NVC):
                    pt = ps_sc.tile([128, 64], F32, tag="sc")
                    nc.tensor.transpose(
                        pt,
                        vT[64 * h : 64 * h + 64, j * 128 : (j + 1) * 128],
                        ident[64 * h : 64 * h + 64, :],
                    )
                    nc.vector.tensor_copy(vex[:, h, j, 0:64], pt)

            # ---- causal attention, transposed-scores flash style ----
            # qc-pair blocking with both heads interleaved per k-chunk:
            # keeps up to 4 independent (h, qc) chains in flight so PE
            # streams matmuls back-to-back while ACT runs the exps, and the
            # two heads' K=64 score matmuls land in separate PE row groups
            # (base partitions 0/64) for array-level concurrency.
            yT = ytp.tile([128, T], F32R, tag="yT")
            for qcs in ((0, 1), (2, 3)):
                yts = {}
                for h in range(HPC):
                    for qc in qcs:
                        yts[(h, qc)] = ps_acc.tile(
                            [65, QCH], F32, name=f"yt_h{h}q{qc}", tag="acc"
                        )
                jmax = (qcs[-1] * QCH + QCH) // KCH
                for j in range(jmax):
                    k0 = j * KCH
                    for h in range(HPC):
                        qTh = qT[64 * h : 64 * h + 64, :]
                        kTh = kT[64 * h : 64 * h + 64, :]
                        for qc in qcs:
                            q0 = qc * QCH
                            if k0 >= q0 + QCH:
                                continue
                            nk = (q0 + QCH) // KCH
                            qlo = max(0, k0 - q0)
                            sc = ps_sc.tile([128, QCH], F32, tag="sc")
                            nc.tensor.matmul(
                                sc[:, qlo:QCH],
                                kTh[:, k0 : k0 + KCH],
                                qTh[:, q0 + qlo : q0 + QCH],
                                start=True,
                                stop=True,
                            )
                            ex = expp.tile([128, QCH], F32R, tag="ex")
                            nc.scalar.activation(
                                ex[:, qlo:QCH], sc[:, qlo:QCH], AF.Exp, scale=0.125
                            )
                            if k0 >= q0:
                                # diagonal 128-wide block: zero where k > q
                                nc.gpsimd.affine_select(
                                    out=ex[:, qlo : qlo + 128],
                                    in_=ex[:, qlo : qlo + 128],
                                    compare_op=mybir.AluOpType.is_ge,
                                    fill=0.0,
                                    base=0,
                                    pattern=[[1, 128]],
                                    channel_multiplier=-1,
                                )
                            nc.tensor.matmul(
                                yts[(h, qc)][:, qlo:QCH],
                                vex[:, h, j, :],
                                ex[:, qlo:QCH],
                                start=(j == 0),
                                stop=(j == nk - 1),
                            )
                for (h, qc), yt_ps in yts.items():
                    q0 = qc * QCH
                    # normalize: 1/s = exp(-ln(s)) on ACT (Ln and Exp share the
                    # loaded table set, unlike AF.Reciprocal which forces a
                    # ~1.3us table swap per call), then K=1 broadcast matmul.
                    lns = smallp.tile([1, QCH], F32, tag="lns")
                    nc.scalar.activation(lns, yt_ps[64:65, :], AF.Ln)
                    recip = smallp.tile([1, QCH], F32R, tag="recip")
                    nc.scalar.activation(recip, lns, AF.Exp, scale=-1.0)
                    bc_ps = ps_sc.tile([64, QCH], F32, tag="sc")
                    nc.tensor.matmul(bc_ps, ones_row, recip, start=True, stop=True)
                    bc_sb2 = smallp.tile([64, QCH], F32, tag="bcast")
                    nc.vector.tensor_copy(bc_sb2, bc_ps)
                    nc.vector.tensor_mul(
                        yT[64 * h : 64 * h + 64, q0 : q0 + QCH],
                        yt_ps[0:64, :],
                        bc_sb2,
                    )

            # ---- output projection (partial over this core's 128 dims) ----
            for tcb in range(T // 128):
                for g in range(2):
                    ps = ps_acc.tile([128, 512], F32, tag="acc")
                    nc.tensor.matmul(
                        ps,
                        yT[:, tcb * 128 : (tcb + 1) * 128],
                        wp_sb[:, g * 512 : (g + 1) * 512],
                        start=True,
                        stop=True,
                    )
                    ot = outp.tile([128, 512], F32, tag="ot")
                    if g == 0:
                        nc.vector.tensor_copy(ot, ps)
                    else:
                        nc.scalar.copy(ot, ps)
                    nc.sync.dma_start(
                        out=out_d.ap()[
                            t0 + tcb * 128 : t0 + (tcb + 1) * 128,
                            g * 512 : (g + 1) * 512,
                        ],
                        in_=ot,
                    )

    _split_multi_waits(nc)
    return nc


_NC_CACHE = None


def _get_nc():
    global _NC_CACHE
    if _NC_CACHE is None:
        _NC_CACHE = build_kernel()
    return _NC_CACHE


def kernel_with_results(x, W_attn, b_attn, W_proj, b_proj, trace=False):
    x = np.asarray(x, dtype=np.float32)
    W_attn = np.asarray(W_attn, dtype=np.float32)
    b_attn = np.asarray(b_attn, dtype=np.float32)
    W_proj = np.asarray(W_proj, dtype=np.float32)
    b_proj = np.asarray(b_proj, dtype=np.float32)

    xT = np.ascontiguousarray(x.reshape(BT, C).T)  # [C, BT]
    in_maps = []
    for c in range(N_CORES):
        lo = c * DPC
        wc = np.ascontiguousarray(
            np.concatenate(
                [
                    W_attn[:, lo : lo + DPC],
                    W_attn[:, C + lo : C + lo + DPC],
                    W_attn[:, 2 * C + lo : 2 * C + lo + DPC],
                ],
                axis=1,
            )
        )
        bc = np.ascontiguousarray(
            np.stack(
                [
                    b_attn[lo : lo + DPC],
                    b_attn[C + lo : C + lo + DPC],
                    b_attn[2 * C + lo : 2 * C + lo + DPC],
                ]
            ).reshape(3, DPC, 1)
        )
        wp = np.ascontiguousarray(W_proj[lo : lo + DPC, :])
        in_maps.append({"xT": xT, "wc": wc, "bc": bc, "wp": wp})

    nc = _get_nc()
    res = run_bass_kernel_spmd(
        nc, in_maps, core_ids=list(range(N_CORES)), trace=trace
    )
    acc = np.zeros((BT, C), dtype=np.float64)
    for c in range(N_CORES):
        acc += res.results[c]["out"].astype(np.float64)
    out = (acc + b_proj.astype(np.float64)).astype(np.float32)
    return out.reshape(B, T, C), res


def kernel(x, W_attn, b_attn, W_proj, b_proj):
    out, _ = kernel_with_results(x, W_attn, b_attn, W_proj, b_proj)
    return out


# revision 19
# speedup vs baseline: 1.1376x; 1.0395x over previous
"""Causal self-attention (B=4, T=2048, C=1024, NH=16) on 8 TRN2 NeuronCores.

Sharding: tensor-parallel over heads — 2 heads per core. Each core computes
its slice of qkv (transposed layout), full causal attention for its heads,
and a partial output projection; the host sums the 8 partials and adds b_proj.

Matmuls run in float32r (fp32 bits, reduced-precision PE mode, ~1.6e-4 rel
err) which streams at bf16 speed for free dims >= 256.

Layout notes:
 - qkv is computed transposed (qT/kT/vT: [dim, tok]) so scores can be formed
   as scoresT[k, q] = kT.T @ qT with d on partitions; softmax runs over the
   partition (k) axis using exp on ACT, a ones-column in the p@v matmul for
   the denominator, and a K=1 broadcast matmul for the reciprocal.
 - x is transposed on the host once (layout prep during sharding) so qkv
   needs no on-device transposes; v is re-transposed to natural layout on PE.
"""

import sys

import numpy as np

try:
    import concourse.bass as bass
except ImportError:  # grading container may not have it on sys.path
    sys.path.insert(0, "/opt/trn_rl_repo")
    import concourse.bass as bass

from contextlib import ExitStack

import concourse.mybir as mybir
import concourse.tile as tile
from concourse.bass_utils import run_bass_kernel_spmd


B, T, C, NH, HD = 4, 2048, 1024, 16, 64
N_CORES = 8
HPC = NH // N_CORES  # heads per core = 2
DPC = HPC * HD  # dims per core = 128
BT = B * T  # 8192
QCH = 512  # q-chunk (moving free dim)
KCH = 128  # k-chunk (contraction tile)
TCH = 512  # token chunk for qkv
F32 = mybir.dt.float32
F32R = mybir.dt.float32r
AF = mybir.ActivationFunctionType


def _r(ap):
    return ap.bitcast(F32R)


def _act_reciprocal(nc, out, in_):
    """Reciprocal on the scalar engine (~430ns for [1,512] vs ~3.3us for
    nc.vector.reciprocal's Newton chain). bass blocks AF.Reciprocal behind an
    accuracy warning; the spline is good to ~1e-5 rel which is far below this
    kernel's f32r noise floor, so emit the instruction directly."""
    eng = nc.scalar
    ins = [
        eng.lower_ap(in_),
        mybir.ImmediateValue(dtype=mybir.dt.float32, value=0.0),
        mybir.ImmediateValue(dtype=mybir.dt.float32, value=1.0),
        mybir.ImmediateValue(dtype=mybir.dt.float32, value=0.0),
    ]
    return eng.add_instruction(
        mybir.InstActivation(
            name=nc.get_next_instruction_name(),
            func=AF.Reciprocal,
            ins=ins,
            outs=[eng.lower_ap(out)],
        )
    )


def _split_multi_waits(nc):
    """Walrus in this container accepts only ONE sync wait per instruction.
    Hoist extra waits onto same-engine NoOps inserted just before."""
    n = 0
    for f in nc.m.functions:
        for b in f.blocks:
            insts = b.instructions
            if not any(
                i.sync_info is not None
                and i.sync_info.on_wait
                and len(i.sync_info.on_wait) > 1
                for i in insts
            ):
                continue
            new = []
            for ins in insts:
                si = ins.sync_info
                if si is not None and si.on_wait and len(si.on_wait) > 1:
                    waits = list(si.on_wait)
                    for w in waits[:-1]:
                        nop = mybir.InstNoOp(
                            name=f"{ins.name}-ws{n}", ins=[], outs=[]
                        )
                        nop.engine = ins.engine
                        nop.bass_nofuse = True
                        nop.sync_info = mybir.SyncInfo(on_wait=[w], on_update=[])
                        if ins.debug is not None:
                            nop.debug = ins.debug
                        new.append(nop)
                        n += 1
                    ins.sync_info = mybir.SyncInfo(
                        on_wait=[waits[-1]], on_update=list(si.on_update or [])
                    )
                new.append(ins)
            b.instructions = new
    return n


def build_kernel():
    nc = bass.Bass("TRN2", target_bir_lowering=False, debug=False, num_devices=N_CORES)
    xT_d = nc.dram_tensor("xT", [C, BT], F32R, kind="ExternalInput")
    wc_d = nc.dram_tensor("wc", [C, 3 * DPC], F32R, kind="ExternalInput")
    bc_d = nc.dram_tensor("bc", [3, DPC, 1], F32, kind="ExternalInput")
    wp_d = nc.dram_tensor("wp", [DPC, C], F32R, kind="ExternalInput")
    out_d = nc.dram_tensor("out", [BT, C], F32, kind="ExternalOutput")

    with tile.TileContext(nc) as tc, ExitStack() as ctx:
        consts = ctx.enter_context(tc.tile_pool(name="consts", bufs=1))
        xpool = ctx.enter_context(tc.tile_pool(name="x", bufs=16))
        qkvp = ctx.enter_context(tc.tile_pool(name="qkv", bufs=2))
        vexp = ctx.enter_context(tc.tile_pool(name="vext", bufs=2))
        ytp = ctx.enter_context(tc.tile_pool(name="yt", bufs=2))
        expp = ctx.enter_context(tc.tile_pool(name="expt", bufs=6))
        smallp = ctx.enter_context(tc.tile_pool(name="small", bufs=4))
        outp = ctx.enter_context(tc.tile_pool(name="outt", bufs=4))
        ps_acc = ctx.enter_context(tc.tile_pool(name="ps_acc", bufs=5, space="PSUM"))
        ps_sc = ctx.enter_context(tc.tile_pool(name="ps_sc", bufs=3, space="PSUM"))

        # [128, 64] tile holding I64 in partitions 0-63 and again in 64-127,
        # so each head's vT slice has an identity at its own base partition.
        ident = consts.tile([128, 64], F32)
        nc.gpsimd.memset(ident, 0.0)
        for half in range(2):
            nc.gpsimd.affine_select(
                out=ident[64 * half : 64 * half + 64, :],
                in_=ident[64 * half : 64 * half + 64, :],
                compare_op=mybir.AluOpType.not_equal,
                fill=1.0,
                base=0,
                pattern=[[-1, 64]],
                channel_multiplier=1,
            )
        ones_row = consts.tile([1, 64], F32R)
        nc.vector.memset(ones_row.bitcast(F32), 1.0)

        # weights: wc [1024, 384] -> [128, 8, 384] (kc chunks on free dim)
        w_sb = consts.tile([128, 8, 3 * DPC], F32R)
        nc.sync.dma_start(
            out=w_sb, in_=wc_d.ap().rearrange("(kc p) c -> p kc c", p=128)
        )
        wp_sb = consts.tile([128, C], F32R)
        nc.sync.dma_start(out=wp_sb, in_=wp_d.ap())
        bc_sb = consts.tile([128, 3], F32)
        nc.sync.dma_start(out=bc_sb, in_=bc_d.ap().rearrange("g p one -> p (g one)"))

        NKC = C // 128  # 8 contraction chunks for qkv
        NTC = T // TCH  # 4 token chunks per batch
        NQC = T // QCH  # 4 q-chunks per batch (per head)
        NVC = T // 128  # 16 v chunks per batch

        for b in range(B):
            t0 = b * T
            # ---- qkv (transposed): qT/kT/vT [128, T] for this batch ----
            qT = qkvp.tile([128, T], F32R, tag="qT")
            kT = qkvp.tile([128, T], F32R, tag="kT")
            vT = qkvp.tile([128, T], F32, tag="vT")
            dest = [qT, kT, vT]
            for tcb in range(NTC):
                xts = []
                for kc in range(NKC):
                    xt = xpool.tile([128, TCH], F32R, tag="xt")
                    nc.sync.dma_start(
                        out=xt,
                        in_=xT_d.ap()[
                            kc * 128 : (kc + 1) * 128,
                            t0 + tcb * TCH : t0 + (tcb + 1) * TCH,
                        ],
                    )
                    xts.append(xt)
                for g in range(3):
                    ps = ps_acc.tile([128, TCH], F32, tag="acc")
                    for kc in range(NKC):
                        nc.tensor.matmul(
                            ps,
                            w_sb[:, kc, g * 128 : (g + 1) * 128],
                            xts[kc],
                            start=(kc == 0),
                            stop=(kc == NKC - 1),
                        )
                    # psum -> sbuf with bias add (b_attn slice, per-partition);
                    # on DVE to keep ACT free for the attention exps
                    nc.vector.tensor_scalar_add(
                        dest[g][:, tcb * TCH : (tcb + 1) * TCH],
                        ps,
                        bc_sb[:, g : g + 1],
                    )

            # ---- v back to natural layout, with ones column: [128, 65] ----
            vex = vexp.tile([128, HPC, NVC, 65], F32R, tag="vex")
            nc.vector.memset(vex[:, :, :, 64:65].bitcast(F32), 1.0)
            for h in range(HPC):
                for j in range(NVC):
                    pt = ps_sc.tile([128, 64], F32, tag="sc")
                    nc.tensor.transpose(
                        pt,
                        vT[64 * h : 64 * h + 64, j * 128 : (j + 1) * 128],
                        ident[64 * h : 64 * h + 64, :],
                    )
                    nc.vector.tensor_copy(vex[:, h, j, 0:64], pt)

            # ---- causal attention, transposed-scores flash style ----
            # qc-pair blocking with both heads interleaved per k-chunk:
            # keeps up to 4 independent (h, qc) chains in flight so PE
            # streams matmuls back-to-back while ACT runs the exps, and the
            # two heads' K=64 score matmuls land in separate PE row groups
            # (base partitions 0/64) for array-level concurrency.
            yT = ytp.tile([128, T], F32R, tag="yT")
            for h in range(HPC):
                qTh = qT[64 * h : 64 * h + 64, :]
                kTh = kT[64 * h : 64 * h + 64, :]
                yts = {}
                for qc in range(NQC):
                    yts[(h, qc)] = ps_acc.tile(
                        [65, QCH], F32, name=f"yt_h{h}q{qc}", tag="acc"
                    )
                for j in range(NVC):
                    k0 = j * KCH
                    exs = {}
                    for qc in range(NQC):
                        q0 = qc * QCH
                        if k0 >= q0 + QCH:
                            continue
                        qlo = max(0, k0 - q0)
                        sc = ps_sc.tile([128, QCH], F32, tag="sc")
                        nc.tensor.matmul(
                            sc[:, qlo:QCH],
                            kTh[:, k0 : k0 + KCH],
                            qTh[:, q0 + qlo : q0 + QCH],
                            start=True,
                            stop=True,
                        )
                        ex = expp.tile([128, QCH], F32R, tag="ex")
                        nc.scalar.activation(
                            ex[:, qlo:QCH], sc[:, qlo:QCH], AF.Exp, scale=0.125
                        )
                        if k0 >= q0:
                            # diagonal 128-wide block: zero where k > q
                            nc.gpsimd.affine_select(
                                out=ex[:, qlo : qlo + 128],
                                in_=ex[:, qlo : qlo + 128],
                                compare_op=mybir.AluOpType.is_ge,
                                fill=0.0,
                                base=0,
                                pattern=[[1, 128]],
                                channel_multiplier=-1,
                            )
                        exs[qc] = (ex, qlo)
                    for qc, (ex, qlo) in exs.items():
                        q0 = qc * QCH
                        nk = (q0 + QCH) // KCH
                        nc.tensor.matmul(
                            yts[(h, qc)][:, qlo:QCH],
                            vex[:, h, j, :],
                            ex[:, qlo:QCH],
                            start=(j == 0),
                            stop=(j == nk - 1),
                        )
                for (h, qc), yt_ps in yts.items():
                    q0 = qc * QCH
                    # normalize: 1/s = exp(-ln(s)) on ACT (Ln and Exp share the
                    # loaded table set, unlike AF.Reciprocal which forces a
                    # ~1.3us table swap per call), then K=1 broadcast matmul.
                    lns = smallp.tile([1, QCH], F32, tag="lns")
                    nc.scalar.activation(lns, yt_ps[64:65, :], AF.Ln)
                    recip = smallp.tile([1, QCH], F32R, tag="recip")
                    nc.scalar.activation(recip, lns, AF.Exp, scale=-1.0)
                    bc_ps = ps_sc.tile([64, QCH], F32, tag="sc")
                    nc.tensor.matmul(bc_ps, ones_row, recip, start=True, stop=True)
                    bc_sb2 = smallp.tile([64, QCH], F32, tag="bcast")
                    nc.vector.tensor_copy(bc_sb2, bc_ps)
                    nc.vector.tensor_mul(
                        yT[64 * h : 64 * h + 64, q0 : q0 + QCH],
                        yt_ps[0:64, :],
                        bc_sb2,
                    )

            # ---- output projection (partial over this core's 128 dims) ----
            for tcb in range(T // 128):
                for g in range(2):
                    ps = ps_acc.tile([128, 512], F32, tag="acc")
                    nc.tensor.matmul(
                        ps,
                        yT[:, tcb * 128 : (tcb + 1) * 128],
                        wp_sb[:, g * 512 : (g + 1) * 512],
                        start=True,
                        stop=True,
                    )
                    ot = outp.tile([128, 512], F32, tag="ot")
                    if g == 0:
                        nc.vector.tensor_copy(ot, ps)
                    else:
                        nc.scalar.copy(ot, ps)
                    nc.sync.dma_start(
                        out=out_d.ap()[
                            t0 + tcb * 128 : t0 + (tcb + 1) * 128,
                            g * 512 : (g + 1) * 512,
                        ],
                        in_=ot,
                    )

    _split_multi_waits(nc)
    return nc


_NC_CACHE = None


def _get_nc():
    global _NC_CACHE
    if _NC_CACHE is None:
        _NC_CACHE = build_kernel()
    return _NC_CACHE


def kernel_with_results(x, W_attn, b_attn, W_proj, b_proj, trace=False):
    x = np.asarray(x, dtype=np.float32)
    W_attn = np.asarray(W_attn, dtype=np.float32)
    b_attn = np.asarray(b_attn, dtype=np.float32)
    W_proj = np.asarray(W_proj, dtype=np.float32)
    b_proj = np.asarray(b_proj, dtype=np.float32)

    xT = np.ascontiguousarray(x.reshape(BT, C).T)  # [C, BT]
    in_maps = []
    for c in range(N_CORES):
        lo = c * DPC
        wc = np.ascontiguousarray(
            np.concatenate(
                [
                    W_attn[:, lo : lo + DPC],
                    W_attn[:, C + lo : C + lo + DPC],
                    W_attn[:, 2 * C + lo : 2 * C + lo + DPC],
                ],
                axis=1,
            )
        )
        bc = np.ascontiguousarray(
            np.stack(
                [
                    b_attn[lo : lo + DPC],
                    b_attn[C + lo : C + lo + DPC],
                    b_attn[2 * C + lo : 2 * C + lo + DPC],
                ]
            ).reshape(3, DPC, 1)
        )
        wp = np.ascontiguousarray(W_proj[lo : lo + DPC, :])
        in_maps.append({"xT": xT, "wc": wc, "bc": bc, "wp": wp})

    nc = _get_nc()
    res = run_bass_kernel_spmd(
        nc, in_maps, core_ids=list(range(N_CORES)), trace=trace
    )
    acc = np.zeros((BT, C), dtype=np.float64)
    for c in range(N_CORES):
        acc += res.results[c]["out"].astype(np.float64)
    out = (acc + b_proj.astype(np.float64)).astype(np.float32)
    return out.reshape(B, T, C), res


def kernel(x, W_attn, b_attn, W_proj, b_proj):
    out, _ = kernel_with_results(x, W_attn, b_attn, W_proj, b_proj)
    return out
